# revision 4
# baseline (speedup 1.0000x reference)
"""Causal multi-head attention (QKV-packed) on 8 Trainium2 NeuronCores.

Sharding: pure head-parallel. B*H = 32 (batch, head) pairs -> 4 per core,
zero inter-core communication. Per head, flash-style causal attention in
the transposed [k, q] orientation (no on-device transposes). Key design
points, in rough order of impact:

  - Scores land in three rotating 2-bank PSUM "group" tiles (1024 f32
    cols each), packed GAPLESSLY by a greedy packer that reorders each
    strip's diagonal partials (384/128/128/128) to match the current
    bank phase -> every group is ONE contiguous exp instruction.
  - The causal mask is folded into the raw scores ON THE PE: for each
    diagonal piece a second matmul accumulates -BIG * tri_gt (stationary
    -BIG*I, moving 0/1 upper-triangle constant) into the piece's first
    128 columns, so exp emits exact zeros there. No mask multiplies
    exist anywhere downstream.
  - Strictly-below-diagonal 512-wide blocks (everything except strip 0
    and head 0's warmup strips) compute scores in fp8e4 with the
    DoubleRow perf mode: Q^T/K^T are pre-split on the host into
    [64, 2, S] (d = p + 64r), and the PE contracts 64 partitions x 2
    rows at 0.5 cycles/col -- 2x bf16 speed. PE total drops to ~49us.
  - The exp work (~58us at the ACT engine's 0.833ns/col) exceeds any
    single engine, so it is SPLIT: the ACT table exp handles strip-0
    groups (short causal rows need exact weights) and ~2/3 of the rest;
    every 3rd clean group runs a Schraudolph bit-trick exp on the DVE
    instead: i16 = trunc(score * SCALE * 128*log2(e) + 16250.5) written
    through an int16-bitcast view and re-read as bf16 (rel err +-3.5%,
    which softmax-normalizes away over >=512 keys; verified 6.9e-3
    end-to-end vs the 2e-2 budget).
  - The softmax denominator is NOT reduced on-device: full blocks
    elementwise-accumulate into a bf16 acc[128,512] on the DVE (2x
    mode), the diagonal partials into an INDEPENDENT acc2[128,384] on
    the otherwise-idle Pool engine (GPSIMD cannot touch PSUM, but these
    are SBUF-to-SBUF), and the host does the final 128-partition sum
    and the divide. This deletes the PE ones-matmuls and the DVE
    reciprocal/normalize entirely.
  - Per strip, unnormalized O^T (PSUM f32) is evacuated to SBUF bf16 by
    whichever of ACT (Activation-Copy) / DVE is less loaded, and ships
    with acc+acc2 in ONE store DMA (obT strip layout [o|acc|acc2]).
  - Strip epilogues are deferred one consume cycle so their PSUM
    evacuation is always ready when issued (the per-engine queues
    complete in order; a long-waiting instruction poisons its engine's
    completion counter for every cross-engine dependent).
  - The tri_gt / -BIG*I constants ride in the first 256 columns of head
    0's qkvT so the critical first load is small; the first two loads
    dispatch via the Pool sequencer (SWDGE) to stay off the serialized
    HWDGE queue.
"""

import sys

if "/opt/trn_rl_repo" not in sys.path:
    sys.path.insert(0, "/opt/trn_rl_repo")

import numpy as np

B, S, H, D = 2, 2048, 16, 128
NCORES = 8
HPC = (B * H) // NCORES  # heads per core = 4
QS = 512   # q-strip width
KB = 128   # k-block (partition dim)
SCALE = 1.0 / float(np.sqrt(D))
# Schraudolph exp in bf16 bit-space: i16 = trunc(y*EXA*SCALE + EXB) viewed
# as bf16 ~= exp(y*SCALE), rel err within +-3.5% (validated vs np.exp).
# Only ever used for strictly-below-diagonal score blocks, where softmax
# weight jitter averages out over >=512 keys.
EXA = 184.6627   # 128*log2(e)
EXB = 16250.5
NSTRIP = S // QS  # 4
BANK = 512  # PSUM bank width in f32 elements
SW = 2 * QS + 384  # per-strip obT stride: o | acc | acc2 | acc3
QOFF = 256  # cst block packed before Q in qkvT
CAP_ = {2: 1536, 3: 1024}
CAP = None  # set in module init below

_nc_cache = {}

import os
CFG_RING = int(os.environ.get("K_RING", "3"))        # sg tags
CFG_DVEBIAS = float(os.environ.get("K_DVEBIAS", "0"))  # extra est bias vs dve exp



def strip_blocks(s):
    """Blocks of strip s: (j, off, w, diag). Fulls j=0..4s (j=4s is
    full-width but causally masked in its first 128 columns); partials
    tile [128,512) of the strip: j=4s+1 at off 128 (w 384), j=4s+2 split
    into a masked 128 (off 256) and a clean 128 (off 384), j=4s+3 at off
    384 (w 128). All masked blocks have the tri pattern in their first
    128 columns."""
    fulls = [(j, 0, 512, j == 4 * s) for j in range(4 * s + 1)]
    parts = [
        (4 * s + 1, 128, 384, True),
        (4 * s + 3, 384, 128, True),
        (4 * s + 2, 256, 128, True),
        (4 * s + 2, 384, 128, False),
    ]
    return fulls, parts


def build_stream(strip_orders):
    """Greedy gapless packer. Returns groups: (used, blocks) with blocks
    = (col, h, s, j, off, w, diag, prefetch, strip_done). Within a strip
    block order is free, so at a non-bank-aligned phase we place 128/384
    partials first; every placement is 128-aligned and never crosses a
    bank, so each group is a single contiguous span."""
    groups = []
    col = 0
    cur = []

    def place(ent, w):
        nonlocal col, cur
        if col + w > CAP:
            groups.append((col, cur))
            col, cur = 0, []
        cur.append((col,) + ent)
        col += w

    for h in range(HPC):
        for si, s in enumerate(strip_orders[h]):
            fulls, parts = strip_blocks(s)
            fulls = list(fulls)
            parts = sorted(parts, key=lambda p: -p[2])  # 384 first
            pf = si == 1 and h + 1 < HPC
            emitted = []
            while fulls or parts:
                rem = BANK - (col % BANK)
                if rem == BANK and fulls:
                    j, off, w, diag = fulls.pop(0)
                elif parts and parts[0][2] <= rem:
                    j, off, w, diag = parts.pop(0)
                elif parts and parts[-1][2] <= rem:
                    j, off, w, diag = parts.pop()
                else:
                    # nothing fits the bank remainder (no partials left);
                    # fulls need a fresh bank: only possible at rem==BANK,
                    # so this is unreachable with the 384/128*3 multiset
                    j, off, w, diag = fulls.pop(0)
                emitted.append((h, s, j, off, w, diag, pf and not emitted))
                place(emitted[-1] + (False,), w)
            # mark strip_done on the last placed block of this strip
            lc = cur[-1]
            cur[-1] = lc[:-1] + (True,)
    groups.append((col, cur))
    return groups


def _build_nc():
    import concourse.bass as bass  # noqa: F401
    import concourse.mybir as mybir
    from concourse import bacc
    from concourse.tile import TileContext

    f32 = mybir.dt.float32
    bf16 = mybir.dt.bfloat16
    i16 = mybir.dt.int16
    f8 = mybir.dt.float8e4
    Exp = mybir.ActivationFunctionType.Exp
    Copy = mybir.ActivationFunctionType.Copy
    Mult = mybir.AluOpType.mult
    Add = mybir.AluOpType.add
    DoubleRow = mybir.MatmulPerfMode.DoubleRow

    nc = bacc.Bacc()
    # One packed input per head, bf16 [128, 3*S]: cols [0,S) = Q^T,
    # [S,2S) = K^T, [2S,3S) = V swizzled (v[p, j*KB+d] = V[j*KB+p, d]).
    qkvT = nc.declare_dram_parameter(
        "qkvT", [HPC, 128, 3 * S + 256], bf16, isOutput=False
    )
    # fp8 copy of Q^T/K^T for DoubleRow score matmuls on clean (strictly
    # below-diagonal) 512-wide blocks: [64, 2, 2S] with row r holding
    # d = p + 64*r; cols [0,S) = Q^T, [S,2S) = K^T. The PE contracts the
    # (64 partitions x 2 rows) pair at 0.5 cycles/col -- 2x bf16 speed.
    qk8 = nc.declare_dram_parameter("qk8", [HPC, 64, 2, 2 * S], f8, isOutput=False)
    # Per head and strip s (stride SW): [o(512) | acc(512) | acc2(384)]:
    # o = unnormalized O^T strip; acc = exp-sums of the full k-blocks
    # (DVE-owned); acc2 = exp-sums of the partial diagonal pieces
    # (Pool-owned, q-offset 128..512). Host sums partitions of acc +
    # acc2 for the softmax denominator.
    obT = nc.declare_dram_parameter(
        "obT", [HPC, 128, NSTRIP * SW], bf16, isOutput=True
    )

    # First head starts with the tiny strip 0 (fast pipeline fill); the
    # final head ends on strip 0 so the epilogue after the last exp is
    # small.
    _SO = {
        "a": [[0, 1, 2, 3], [3, 2, 1, 0], [0, 1, 2, 3], [3, 2, 1, 0]],
        "b": [[0, 2, 1, 3], [3, 1, 2, 0], [0, 2, 1, 3], [3, 1, 2, 0]],
        "c": [[0, 1, 2, 3], [2, 3, 1, 0], [1, 3, 2, 0], [3, 2, 1, 0]],
        "d": [[0, 3, 1, 2], [2, 1, 3, 0], [0, 3, 1, 2], [2, 1, 3, 0]],
        "e": [[0, 1, 3, 2], [3, 2, 1, 0], [1, 2, 3, 0], [3, 2, 1, 0]],
    }
    strip_orders = _SO[os.environ.get("K_SO", "a")]
    tail_strips = {(HPC - 1, strip_orders[-1][-1]), (HPC - 1, strip_orders[-1][-2])}
    groups = build_stream(strip_orders)

    with TileContext(nc) as tc:
        with (
            nc.allow_low_precision(
                reason="bf16 P/V/acc; softmax weights tolerate 2^-9"
            ),
            tc.tile_pool(name="cpool", bufs=1) as cpool,
            tc.tile_pool(name="qkpool", bufs=int(os.environ.get("K_QKB","2"))) as qkpool,
            tc.tile_pool(name="qk8pool", bufs=int(os.environ.get("K_QKB","2"))) as qk8pool,
            tc.tile_pool(name="ptpool", bufs=int(os.environ.get("K_PTB","3"))) as ptpool,
            tc.tile_pool(name="obpool", bufs=int(os.environ.get("K_OBB","6"))) as obpool,
            tc.tile_pool(name="psg", bufs=1, space="PSUM") as psg,
            tc.tile_pool(name="pso", bufs=2, space="PSUM") as pso,
        ):
            cst_sb = cpool.tile([128, 256], bf16)
            tri_gt = cst_sb[:, 0:128]
            neg_eye = cst_sb[:, 128:256]

            qkv_tiles = {}
            qk8_tiles = {}

            def load_head(hh, first=False):
                t = qkpool.tile([128, 3 * S + 256], bf16, tag="qkv_sb")
                t8 = qk8pool.tile([64, 2, 2 * S], f8, tag="qk8_sb")
                qkv_tiles[hh] = t
                qk8_tiles[hh] = t8
                if first:
                    # split the first head's load so the first exp fires
                    # as early as possible: ONE chunk with [cst | Q strip
                    # 0] then K block 0-3, then the rest in need order;
                    # head 0's early strips (0,1) use bf16-only scores so
                    # qk8 may trail. All qkvT offsets carry QOFF=256.
                    nc.gpsimd.dma_start(out=cst_sb[:], in_=qkvT[hh][:, 0:256])
                    nc.gpsimd.dma_start(
                        out=t[:, QOFF : QOFF + 512],
                        in_=qkvT[hh][:, QOFF : QOFF + 512],
                    )
                    nc.sync.dma_start(
                        out=t[:, QOFF + S : QOFF + S + 512],
                        in_=qkvT[hh][:, QOFF + S : QOFF + S + 512],
                    )
                    nc.scalar.dma_start(
                        out=t[:, QOFF + 512 : QOFF + 1024],
                        in_=qkvT[hh][:, QOFF + 512 : QOFF + 1024],
                    )
                    nc.sync.dma_start(
                        out=t[:, QOFF + S + 512 : QOFF + S + 1024],
                        in_=qkvT[hh][:, QOFF + S + 512 : QOFF + S + 1024],
                    )
                    for c0, c1 in (
                        (QOFF + 2 * S, QOFF + 2 * S + 1024),
                        (QOFF + S + 1024, QOFF + 2 * S),
                        (QOFF + 1024, QOFF + S),
                    ):
                        nc.sync.dma_start(out=t[:, c0:c1], in_=qkvT[hh][:, c0:c1])
                    nc.sync.dma_start(out=t8[:, :, 0:S], in_=qk8[hh][:, :, 0:S])
                    nc.sync.dma_start(
                        out=t8[:, :, S : 2 * S], in_=qk8[hh][:, :, S : 2 * S]
                    )
                    nc.sync.dma_start(
                        out=t[:, QOFF + 2 * S + 1024 : QOFF + 3 * S],
                        in_=qkvT[hh][:, QOFF + 2 * S + 1024 : QOFF + 3 * S],
                    )
                else:
                    # non-first heads touch bf16 Q/K only for strip 0
                    # (everything else is fp8), so load just those 512
                    # columns; fp8 Q/K early (the fresh head's first
                    # strip opens with clean fp8 fulls)
                    nc.sync.dma_start(out=t8[:, :, 0:S], in_=qk8[hh][:, :, 0:S])
                    nc.sync.dma_start(
                        out=t8[:, :, S : 2 * S], in_=qk8[hh][:, :, S : 2 * S]
                    )
                    nc.sync.dma_start(
                        out=t[:, QOFF + S : QOFF + S + 512],
                        in_=qkvT[hh][:, QOFF + S : QOFF + S + 512],
                    )
                    nc.sync.dma_start(
                        out=t[:, QOFF : QOFF + 512],
                        in_=qkvT[hh][:, QOFF : QOFF + 512],
                    )
                    nc.sync.dma_start(
                        out=t[:, QOFF + 2 * S : QOFF + 3 * S],
                        in_=qkvT[hh][:, QOFF + 2 * S : QOFF + 3 * S],
                    )

            strip_states = {}

            def get_state(h, s):
                if (h, s) not in strip_states:
                    o_ps = pso.tile([128, QS], f32, tag="o_ps")
                    ob = obpool.tile([128, SW], bf16, tag="ob")
                    est["dve"] += (4 * s + 1) * 327
                    est["pool"] += 1900
                    strip_states[(h, s)] = {
                        "o_ps": o_ps, "ob": ob, "acc": ob[:, QS : 2 * QS],
                        "acc2": ob[:, 2 * QS : SW],
                        "first": True, "cover2": 384,
                        "ocount": 0, "nmm": 4 * s + 5,
                    }
                return strip_states[(h, s)]

            pending_fin = []

            def emit_fins():
                while pending_fin:
                    h, s, stt, tail = pending_fin.pop(0)
                    c0 = SW * s
                    if tail:
                        nc.sync.dma_start(
                            out=obT[h][:, c0 + QS : c0 + SW],
                            in_=stt["ob"][:, QS:SW],
                        )
                        nc.scalar.activation(
                            stt["ob"][:, 0:QS], stt["o_ps"][:], Copy
                        )
                        nc.scalar.dma_start(
                            out=obT[h][:, c0 : c0 + QS], in_=stt["ob"][:, 0:QS]
                        )
                    else:
                        if est["act"] < est["dve"]:
                            est["act"] += 612
                            nc.scalar.activation(
                                stt["ob"][:, 0:QS], stt["o_ps"][:], Copy
                            )
                        else:
                            est["dve"] += 658
                            nc.vector.tensor_copy(
                                stt["ob"][:, 0:QS], stt["o_ps"][:]
                            )
                        nc.sync.dma_start(
                            out=obT[h][:, c0 : c0 + SW], in_=stt["ob"][:]
                        )

            def consume(st, tail=False):
                blocks, pt = st
                emit_fins()

                def do_acc():
                    # exp-sum accumulation; causal masking already
                    # happened in PSUM (bias matmul), so P is exact.
                    # Full 512-blocks chain into acc on the DVE; partial
                    # diagonal pieces chain into the INDEPENDENT acc2 on
                    # the otherwise-idle Pool -- the two never touch the
                    # same bytes, so the chains don't serialize.
                    for col, h, s, j, off, w, diag, pf, sd in blocks:
                        stt = strip_states[(h, s)]
                        if w == 512:
                            acc = stt["acc"]
                            if stt["first"]:
                                nc.vector.tensor_copy(acc[:], pt[:, col : col + 512])
                                stt["first"] = False
                            else:
                                nc.vector.tensor_add(
                                    acc[:], acc[:], pt[:, col : col + 512]
                                )
                            continue
                        acc2, lo = stt["acc2"], stt["cover2"]
                        peng = nc.vector if (h, s) in tail_strips else nc.gpsimd
                        o2 = off - 128
                        if o2 >= lo:
                            peng.tensor_add(
                                acc2[:, o2 : o2 + w], acc2[:, o2 : o2 + w],
                                pt[:, col : col + w],
                            )
                        else:
                            cut = min(o2 + w, lo)
                            peng.tensor_copy(
                                acc2[:, o2:cut], pt[:, col : col + (cut - o2)]
                            )
                            if o2 + w > lo:
                                peng.tensor_add(
                                    acc2[:, lo : o2 + w], acc2[:, lo : o2 + w],
                                    pt[:, col + (lo - o2) : col + w],
                                )
                            stt["cover2"] = o2

                def do_o():
                    # O-matmuls (PE)
                    for col, h, s, j, off, w, diag, pf, sd in blocks:
                        stt = strip_states[(h, s)]
                        nc.tensor.matmul(
                            stt["o_ps"][:, off : off + w],
                            lhsT=qkv_tiles[h][
                                :,
                                QOFF + 2 * S + KB * j : QOFF + 2 * S + KB * (j + 1),
                            ],
                            rhs=pt[:, col : col + w],
                            start=stt["ocount"] == 0,
                            stop=stt["ocount"] == stt["nmm"] - 1,
                        )
                        stt["ocount"] += 1

                # at the tail the O->evacuate->store chain is critical:
                # emit it before the acc adds
                if tail:
                    do_o(), do_acc()
                else:
                    do_acc(), do_o()
                # strip epilogue: deferred one consume cycle (see
                # emit_fins) so its PSUM evacuation is ready when issued
                for col, h, s, j, off, w, diag, pf, sd in blocks:
                    if sd:
                        pending_fin.append((h, s, strip_states.pop((h, s)), tail))

            load_head(0, first=True)
            pend_q = []
            # static per-engine busy estimates (ns) steer which engine
            # exponentiates each group: the ACT table exp is exact and
            # mandatory for groups holding causally-masked (diagonal)
            # blocks; clean below-diagonal groups may use the Schraudolph
            # bit-trick exp on DVE (tensor_scalar, ~1.04ns/col) or Pool
            # (~1.39ns/col) to offload the saturated ACT engine.
            est = {"act": 0.0, "dve": 0.0, "pool": 0.0}
            prev_eng = [None]
            ng = len(groups)
            for gi, (used, blocks) in enumerate(groups):
                for col, h, s, j, off, w, diag, pf, sd in blocks:
                    if pf:
                        load_head(h + 1)
                sg = psg.tile([128, CAP], f32, tag=f"sg{gi % CFG_RING}")
                for col, h, s, j, off, w, diag, pf, sd in blocks:
                    get_state(h, s)
                    qkv_sb = qkv_tiles[h]
                    # strip 0 holds the short causal rows where softmax
                    # weight jitter does NOT average out: keep it bf16 +
                    # exact ACT exp. Everything else runs fp8 DoubleRow at
                    # 2x PE speed. (Head 0's strip 1 also stays bf16 so
                    # the fp8 load may trail the critical first groups.)
                    f8ok = s > 0 and (h > 0 or s >= 2)
                    if f8ok:
                        t8 = qk8_tiles[h]
                        nc.tensor.matmul(
                            sg[:, col : col + w],
                            lhsT=t8[:, :, S + KB * j : S + KB * (j + 1)],
                            rhs=t8[:, :, QS * s + off : QS * s + off + w],
                            start=True,
                            stop=not diag,
                            perf_mode=DoubleRow,
                        )
                    else:
                        nc.tensor.matmul(
                            sg[:, col : col + w],
                            lhsT=qkv_sb[
                                :, QOFF + S + KB * j : QOFF + S + KB * (j + 1)
                            ],
                            rhs=qkv_sb[
                                :, QOFF + QS * s + off : QOFF + QS * s + off + w
                            ],
                            start=True,
                            stop=not diag,
                        )
                    if diag:
                        # fold the causal mask into the raw scores: PSUM-
                        # accumulate -BIG * (k_local > q_local) over the
                        # piece's first 128 columns (its triangle), so the
                        # exp emits exact zeros there and no separate mask
                        # multiply is needed anywhere downstream
                        nc.tensor.matmul(
                            sg[:, col : col + 128],
                            lhsT=neg_eye,
                            rhs=tri_gt,
                            start=False,
                            stop=True,
                        )
                pt = ptpool.tile([128, CAP], bf16, tag=f"pt{gi % CFG_RING}")
                # consume older groups BEFORE emitting this group's exp:
                # on an offloaded (DVE/Pool) exp the engine queues are
                # in-order, and an exp still waiting on PE scores must not
                # block ready mask/add side-work queued behind it
                lag = int(os.environ.get("K_LAG", "3")) if gi < ng - 3 else 1
                while len(pend_q) >= lag:
                    consume(pend_q.pop(0), tail=gi >= ng - 3)
                # side work this group induces (adds on DVE, masks + strip
                # evacuation on Pool)
                # offload needs every block's rows to have >=512 keys:
                # exclude strip-0 groups (short rows -> exact ACT exp)
                clean = (all(b[2] > 0 for b in blocks)
                         and gi < ng - int(os.environ.get("K_TAILACT", "3")))
                cost = {
                    "act": used * 0.833 + 185,
                    "dve": used * 1.04 + 125 + CFG_DVEBIAS,
                }  # ns; per-instruction access penalties included
                pat = int(os.environ.get("K_PAT", "3"))
                if clean and pat:
                    eng = "dve" if gi % pat == pat - 1 else "act"
                elif clean:
                    # capacity-aware rotation: back-to-back groups on one
                    # engine serialize the sg ring, so penalize repeats
                    eng = min(
                        cost,
                        key=lambda e: max(
                            est[x] + (cost[x] if x == e else 0)
                            for x in est
                        ) + (300 if e == prev_eng[0] else 0),
                    )
                else:
                    eng = "act"
                prev_eng[0] = eng
                est[eng] += cost[eng]
                if eng == "act":
                    nc.scalar.activation(
                        pt[:, 0:used], sg[:, 0:used], Exp, scale=SCALE
                    )
                else:
                    veng = nc.vector
                    veng.tensor_scalar(
                        pt[:, 0:used].bitcast(i16), sg[:, 0:used],
                        EXA * SCALE, EXB, Mult, Add,
                    )
                pend_q.append((blocks, pt))
            while pend_q:
                consume(pend_q.pop(0), tail=True)
            emit_fins()
    nc.compile()
    return nc


def get_nc():
    if "nc" not in _nc_cache:
        _nc_cache["nc"] = _build_nc()
    return _nc_cache["nc"]


BIG = 50.0 / SCALE  # raw-score causal bias; exp((s-BIG)*SCALE) ~ 0


def _build_const():
    import ml_dtypes

    dk = np.arange(128)[:, None]
    c = np.arange(128)[None, :]
    cst = np.zeros((128, 256), ml_dtypes.bfloat16)
    cst[:, 0:128] = (dk > c).astype(ml_dtypes.bfloat16)
    cst[:, 128:256] = -BIG * np.eye(128, dtype=np.float32)
    return cst


def make_in_maps(qkv):
    import ml_dtypes

    qkv = np.asarray(qkv, dtype=np.float32)
    cst = _build_const()
    in_maps = []
    for core in range(NCORES):
        qkvT = np.empty((HPC, 128, 3 * S + 256), ml_dtypes.bfloat16)
        qk8 = np.empty((HPC, 64, 2, 2 * S), ml_dtypes.float8_e4m3)
        qkvT[:, :, 0:256] = cst
        for i in range(HPC):
            bh = core * HPC + i
            b, h = bh // H, bh % H
            qT = qkv[b, :, 0, h, :].T
            kT = qkv[b, :, 1, h, :].T
            qkvT[i, :, 256 : 256 + S] = qT
            qkvT[i, :, 256 + S : 256 + 2 * S] = kT
            qkvT[i, :, 256 + 2 * S :] = (
                qkv[b, :, 2, h, :]
                .reshape(S // KB, KB, D)
                .transpose(1, 0, 2)
                .reshape(KB, S)
            )
            qk8[i, :, 0, 0:S] = qT[0:64]
            qk8[i, :, 1, 0:S] = qT[64:128]
            qk8[i, :, 0, S:] = kT[0:64]
            qk8[i, :, 1, S:] = kT[64:128]
        in_maps.append({"qkvT": qkvT, "qk8": qk8})
    return in_maps


def assemble_out(results):
    out = np.empty((B, S, H, D), np.float32)
    for core in range(NCORES):
        obc = np.asarray(results[core]["obT"], dtype=np.float32)
        for i in range(HPC):
            bh = core * HPC + i
            b, h = bh // H, bh % H
            ob = obc[i].reshape(128, NSTRIP, SW)
            oT = ob[:, :, 0:QS].reshape(128, S)         # unnormalized O^T
            den = ob[:, :, QS : 2 * QS].sum(axis=0)     # full-block sums
            den[:, 128:] += ob[:, :, 2 * QS :].sum(axis=0)  # partial pieces
            den = den.reshape(S)
            out[b, :, h, :] = (oT / den[None, :]).T
    return out


def kernel(qkv):
    from concourse.bass_utils import run_bass_kernel_spmd

    in_maps = make_in_maps(qkv)
    nc = get_nc()
    res = run_bass_kernel_spmd(nc, in_maps, list(range(NCORES)))
    return assemble_out(res.results)

CAP = CAP_[CFG_RING]


# revision 5
# speedup vs baseline: 1.0178x; 1.0178x over previous
"""Causal multi-head attention (QKV-packed) on 8 Trainium2 NeuronCores.

Sharding: pure head-parallel. B*H = 32 (batch, head) pairs -> 4 per core,
zero inter-core communication. Per head, flash-style causal attention in
the transposed [k, q] orientation (no on-device transposes). Key design
points, in rough order of impact:

  - Scores land in three rotating 2-bank PSUM "group" tiles (1024 f32
    cols each), packed GAPLESSLY by a greedy packer that reorders each
    strip's diagonal partials (384/128/128/128) to match the current
    bank phase -> every group is ONE contiguous exp instruction.
  - The causal mask is folded into the raw scores ON THE PE: for each
    diagonal piece a second matmul accumulates -BIG * tri_gt (stationary
    -BIG*I, moving 0/1 upper-triangle constant) into the piece's first
    128 columns, so exp emits exact zeros there. No mask multiplies
    exist anywhere downstream.
  - Strictly-below-diagonal 512-wide blocks (everything except strip 0
    and head 0's warmup strips) compute scores in fp8e4 with the
    DoubleRow perf mode: Q^T/K^T are pre-split on the host into
    [64, 2, S] (d = p + 64r), and the PE contracts 64 partitions x 2
    rows at 0.5 cycles/col -- 2x bf16 speed. PE total drops to ~49us.
  - The exp work (~58us at the ACT engine's 0.833ns/col) exceeds any
    single engine, so it is SPLIT: the ACT table exp handles strip-0
    groups (short causal rows need exact weights) and ~2/3 of the rest;
    every 3rd clean group runs a Schraudolph bit-trick exp on the DVE
    instead: i16 = trunc(score * SCALE * 128*log2(e) + 16250.5) written
    through an int16-bitcast view and re-read as bf16 (rel err +-3.5%,
    which softmax-normalizes away over >=512 keys; verified 6.9e-3
    end-to-end vs the 2e-2 budget).
  - The softmax denominator is NOT reduced on-device: full blocks
    elementwise-accumulate into a bf16 acc[128,512] on the DVE (2x
    mode), the diagonal partials into an INDEPENDENT acc2[128,384] on
    the otherwise-idle Pool engine (GPSIMD cannot touch PSUM, but these
    are SBUF-to-SBUF), and the host does the final 128-partition sum
    and the divide. This deletes the PE ones-matmuls and the DVE
    reciprocal/normalize entirely.
  - Per strip, unnormalized O^T (PSUM f32) is evacuated to SBUF bf16 by
    whichever of ACT (Activation-Copy) / DVE is less loaded, and ships
    with acc+acc2 in ONE store DMA (obT strip layout [o|acc|acc2]).
  - Strip epilogues are deferred one consume cycle so their PSUM
    evacuation is always ready when issued (the per-engine queues
    complete in order; a long-waiting instruction poisons its engine's
    completion counter for every cross-engine dependent).
  - The tri_gt / -BIG*I constants ride in the first 256 columns of head
    0's qkvT so the critical first load is small; the first two loads
    dispatch via the Pool sequencer (SWDGE) to stay off the serialized
    HWDGE queue.
"""

import sys

if "/opt/trn_rl_repo" not in sys.path:
    sys.path.insert(0, "/opt/trn_rl_repo")

import numpy as np

B, S, H, D = 2, 2048, 16, 128
NCORES = 8
HPC = (B * H) // NCORES  # heads per core = 4
QS = 512   # q-strip width
KB = 128   # k-block (partition dim)
SCALE = 1.0 / float(np.sqrt(D))
# Schraudolph exp in bf16 bit-space: i16 = trunc(y*EXA*SCALE + EXB) viewed
# as bf16 ~= exp(y*SCALE), rel err within +-3.5% (validated vs np.exp).
# Only ever used for strictly-below-diagonal score blocks, where softmax
# weight jitter averages out over >=512 keys.
EXA = 184.6627   # 128*log2(e)
EXB = 16250.5
NSTRIP = S // QS  # 4
BANK = 512  # PSUM bank width in f32 elements
SW = 2 * QS + 384  # per-strip obT stride: o | acc | acc2 | acc3
QOFF = 256  # cst block packed before Q in qkvT
CAP_ = {2: 1536, 3: 1024}
CAP = None  # set in module init below

_nc_cache = {}

import os
CFG_RING = int(os.environ.get("K_RING", "3"))        # sg tags
CFG_DVEBIAS = float(os.environ.get("K_DVEBIAS", "0"))  # extra est bias vs dve exp



def strip_blocks(s):
    """Blocks of strip s: (j, off, w, diag). Fulls j=0..4s (j=4s is
    full-width but causally masked in its first 128 columns); partials
    tile [128,512) of the strip: j=4s+1 at off 128 (w 384), j=4s+2 split
    into a masked 128 (off 256) and a clean 128 (off 384), j=4s+3 at off
    384 (w 128). All masked blocks have the tri pattern in their first
    128 columns."""
    fulls = [(j, 0, 512, j == 4 * s) for j in range(4 * s + 1)]
    parts = [
        (4 * s + 1, 128, 384, True),
        (4 * s + 3, 384, 128, True),
        (4 * s + 2, 256, 128, True),
        (4 * s + 2, 384, 128, False),
    ]
    return fulls, parts


def build_stream(strip_orders):
    """Greedy gapless packer. Returns groups: (used, blocks) with blocks
    = (col, h, s, j, off, w, diag, prefetch, strip_done). Within a strip
    block order is free, so at a non-bank-aligned phase we place 128/384
    partials first; every placement is 128-aligned and never crosses a
    bank, so each group is a single contiguous span."""
    groups = []
    col = 0
    cur = []

    def place(ent, w):
        nonlocal col, cur
        if col + w > CAP:
            groups.append((col, cur))
            col, cur = 0, []
        cur.append((col,) + ent)
        col += w

    for h in range(HPC):
        for si, s in enumerate(strip_orders[h]):
            fulls, parts = strip_blocks(s)
            fulls = list(fulls)
            parts = sorted(parts, key=lambda p: -p[2])  # 384 first
            pf = si == int(os.environ.get("K_PF", "1")) and h + 1 < HPC
            emitted = []
            while fulls or parts:
                rem = BANK - (col % BANK)
                if rem == BANK and fulls:
                    j, off, w, diag = fulls.pop(0)
                elif parts and parts[0][2] <= rem:
                    j, off, w, diag = parts.pop(0)
                elif parts and parts[-1][2] <= rem:
                    j, off, w, diag = parts.pop()
                else:
                    # nothing fits the bank remainder (no partials left);
                    # fulls need a fresh bank: only possible at rem==BANK,
                    # so this is unreachable with the 384/128*3 multiset
                    j, off, w, diag = fulls.pop(0)
                emitted.append((h, s, j, off, w, diag, pf and not emitted))
                place(emitted[-1] + (False,), w)
            # mark strip_done on the last placed block of this strip
            lc = cur[-1]
            cur[-1] = lc[:-1] + (True,)
    groups.append((col, cur))
    return groups


def _build_nc():
    import concourse.bass as bass  # noqa: F401
    import concourse.mybir as mybir
    from concourse import bacc
    from concourse.tile import TileContext

    f32 = mybir.dt.float32
    bf16 = mybir.dt.bfloat16
    i16 = mybir.dt.int16
    f8 = mybir.dt.float8e4
    Exp = mybir.ActivationFunctionType.Exp
    Copy = mybir.ActivationFunctionType.Copy
    Mult = mybir.AluOpType.mult
    Add = mybir.AluOpType.add
    DoubleRow = mybir.MatmulPerfMode.DoubleRow

    nc = bacc.Bacc()
    # One packed input per head, bf16 [128, 3*S]: cols [0,S) = Q^T,
    # [S,2S) = K^T, [2S,3S) = V swizzled (v[p, j*KB+d] = V[j*KB+p, d]).
    qkvT = nc.declare_dram_parameter(
        "qkvT", [HPC, 128, 3 * S + 256], bf16, isOutput=False
    )
    # fp8 copy of Q^T/K^T for DoubleRow score matmuls on clean (strictly
    # below-diagonal) 512-wide blocks: [64, 2, 2S] with row r holding
    # d = p + 64*r; cols [0,S) = Q^T, [S,2S) = K^T. The PE contracts the
    # (64 partitions x 2 rows) pair at 0.5 cycles/col -- 2x bf16 speed.
    qk8 = nc.declare_dram_parameter("qk8", [HPC, 64, 2, 2 * S], f8, isOutput=False)
    # Per head and strip s (stride SW): [o(512) | acc(512) | acc2(384)]:
    # o = unnormalized O^T strip; acc = exp-sums of the full k-blocks
    # (DVE-owned); acc2 = exp-sums of the partial diagonal pieces
    # (Pool-owned, q-offset 128..512). Host sums partitions of acc +
    # acc2 for the softmax denominator.
    obT = nc.declare_dram_parameter(
        "obT", [HPC, 128, NSTRIP * SW], bf16, isOutput=True
    )

    # First head starts with the tiny strip 0 (fast pipeline fill); the
    # final head ends on strip 0 so the epilogue after the last exp is
    # small.
    _SO = {
        "a": [[0, 1, 2, 3], [3, 2, 1, 0], [0, 1, 2, 3], [3, 2, 1, 0]],
        "b": [[0, 2, 1, 3], [3, 1, 2, 0], [0, 2, 1, 3], [3, 1, 2, 0]],
        "c": [[0, 1, 2, 3], [2, 3, 1, 0], [1, 3, 2, 0], [3, 2, 1, 0]],
        "d": [[0, 3, 1, 2], [2, 1, 3, 0], [0, 3, 1, 2], [2, 1, 3, 0]],
        "e": [[0, 1, 3, 2], [3, 2, 1, 0], [1, 2, 3, 0], [3, 2, 1, 0]],
    }
    strip_orders = _SO[os.environ.get("K_SO", "a")]
    tail_strips = {(HPC - 1, strip_orders[-1][-1]), (HPC - 1, strip_orders[-1][-2])}
    groups = build_stream(strip_orders)

    with TileContext(nc) as tc:
        with (
            nc.allow_low_precision(
                reason="bf16 P/V/acc; softmax weights tolerate 2^-9"
            ),
            tc.tile_pool(name="cpool", bufs=1) as cpool,
            tc.tile_pool(name="qkpool", bufs=int(os.environ.get("K_QKB","2"))) as qkpool,
            tc.tile_pool(name="qk8pool", bufs=int(os.environ.get("K_QKB","2"))) as qk8pool,
            tc.tile_pool(name="ptpool", bufs=int(os.environ.get("K_PTB","3"))) as ptpool,
            tc.tile_pool(name="obpool", bufs=int(os.environ.get("K_OBB","6"))) as obpool,
            tc.tile_pool(name="psg", bufs=1, space="PSUM") as psg,
            tc.tile_pool(name="pso", bufs=2, space="PSUM") as pso,
        ):
            cst_sb = cpool.tile([128, 256], bf16)
            tri_gt = cst_sb[:, 0:128]
            neg_eye = cst_sb[:, 128:256]

            qkv_tiles = {}
            qk8_tiles = {}

            def load_head(hh, first=False):
                t = qkpool.tile([128, 3 * S + 256], bf16, tag="qkv_sb")
                t8 = qk8pool.tile([64, 2, 2 * S], f8, tag="qk8_sb")
                qkv_tiles[hh] = t
                qk8_tiles[hh] = t8
                if first:
                    # split the first head's load so the first exp fires
                    # as early as possible: ONE chunk with [cst | Q strip
                    # 0] then K block 0-3, then the rest in need order;
                    # head 0's early strips (0,1) use bf16-only scores so
                    # qk8 may trail. All qkvT offsets carry QOFF=256.
                    nc.gpsimd.dma_start(out=cst_sb[:], in_=qkvT[hh][:, 0:256])
                    nc.gpsimd.dma_start(
                        out=t[:, QOFF : QOFF + 512],
                        in_=qkvT[hh][:, QOFF : QOFF + 512],
                    )
                    nc.sync.dma_start(
                        out=t[:, QOFF + S : QOFF + S + 512],
                        in_=qkvT[hh][:, QOFF + S : QOFF + S + 512],
                    )
                    nc.scalar.dma_start(
                        out=t[:, QOFF + 512 : QOFF + 1024],
                        in_=qkvT[hh][:, QOFF + 512 : QOFF + 1024],
                    )
                    nc.sync.dma_start(
                        out=t[:, QOFF + S + 512 : QOFF + S + 1024],
                        in_=qkvT[hh][:, QOFF + S + 512 : QOFF + S + 1024],
                    )
                    for c0, c1 in (
                        (QOFF + 2 * S, QOFF + 2 * S + 1024),
                        (QOFF + S + 1024, QOFF + 2 * S),
                        (QOFF + 1024, QOFF + S),
                    ):
                        nc.sync.dma_start(out=t[:, c0:c1], in_=qkvT[hh][:, c0:c1])
                    nc.sync.dma_start(out=t8[:, :, 0:S], in_=qk8[hh][:, :, 0:S])
                    nc.sync.dma_start(
                        out=t8[:, :, S : 2 * S], in_=qk8[hh][:, :, S : 2 * S]
                    )
                    nc.sync.dma_start(
                        out=t[:, QOFF + 2 * S + 1024 : QOFF + 3 * S],
                        in_=qkvT[hh][:, QOFF + 2 * S + 1024 : QOFF + 3 * S],
                    )
                else:
                    # non-first heads touch bf16 Q/K only for strip 0
                    # (everything else is fp8), so load just those 512
                    # columns; fp8 Q/K early (the fresh head's first
                    # strip opens with clean fp8 fulls)
                    nc.sync.dma_start(out=t8[:, :, 0:S], in_=qk8[hh][:, :, 0:S])
                    nc.sync.dma_start(
                        out=t8[:, :, S : 2 * S], in_=qk8[hh][:, :, S : 2 * S]
                    )
                    nc.sync.dma_start(
                        out=t[:, QOFF + S : QOFF + S + 512],
                        in_=qkvT[hh][:, QOFF + S : QOFF + S + 512],
                    )
                    nc.sync.dma_start(
                        out=t[:, QOFF : QOFF + 512],
                        in_=qkvT[hh][:, QOFF : QOFF + 512],
                    )
                    nc.sync.dma_start(
                        out=t[:, QOFF + 2 * S : QOFF + 3 * S],
                        in_=qkvT[hh][:, QOFF + 2 * S : QOFF + 3 * S],
                    )

            strip_states = {}

            def get_state(h, s):
                if (h, s) not in strip_states:
                    o_ps = pso.tile([128, QS], f32, tag="o_ps")
                    ob = obpool.tile([128, SW], bf16, tag="ob")
                    est["dve"] += (4 * s + 1) * 327
                    est["pool"] += 1900
                    strip_states[(h, s)] = {
                        "o_ps": o_ps, "ob": ob, "acc": ob[:, QS : 2 * QS],
                        "acc2": ob[:, 2 * QS : SW],
                        "first": True, "cover2": 384,
                        "ocount": 0, "nmm": 4 * s + 5,
                    }
                return strip_states[(h, s)]

            pending_fin = []

            def emit_fins():
                while pending_fin:
                    h, s, stt, tail = pending_fin.pop(0)
                    c0 = SW * s
                    if tail:
                        nc.sync.dma_start(
                            out=obT[h][:, c0 + QS : c0 + SW],
                            in_=stt["ob"][:, QS:SW],
                        )
                        nc.scalar.activation(
                            stt["ob"][:, 0:QS], stt["o_ps"][:], Copy
                        )
                        nc.scalar.dma_start(
                            out=obT[h][:, c0 : c0 + QS], in_=stt["ob"][:, 0:QS]
                        )
                    else:
                        if est["act"] < est["dve"]:
                            est["act"] += 612
                            nc.scalar.activation(
                                stt["ob"][:, 0:QS], stt["o_ps"][:], Copy
                            )
                        else:
                            est["dve"] += 658
                            nc.vector.tensor_copy(
                                stt["ob"][:, 0:QS], stt["o_ps"][:]
                            )
                        nc.sync.dma_start(
                            out=obT[h][:, c0 : c0 + SW], in_=stt["ob"][:]
                        )

            def consume(st, tail=False):
                blocks, pt = st
                emit_fins()

                def do_acc():
                    # exp-sum accumulation; causal masking already
                    # happened in PSUM (bias matmul), so P is exact.
                    # Full 512-blocks chain into acc on the DVE; partial
                    # diagonal pieces chain into the INDEPENDENT acc2 on
                    # the otherwise-idle Pool -- the two never touch the
                    # same bytes, so the chains don't serialize.
                    for col, h, s, j, off, w, diag, pf, sd in blocks:
                        stt = strip_states[(h, s)]
                        if w == 512:
                            acc = stt["acc"]
                            if stt["first"]:
                                nc.vector.tensor_copy(acc[:], pt[:, col : col + 512])
                                stt["first"] = False
                            else:
                                nc.vector.tensor_add(
                                    acc[:], acc[:], pt[:, col : col + 512]
                                )
                            continue
                        acc2, lo = stt["acc2"], stt["cover2"]
                        peng = nc.vector if (h, s) in tail_strips else nc.gpsimd
                        o2 = off - 128
                        if o2 >= lo:
                            peng.tensor_add(
                                acc2[:, o2 : o2 + w], acc2[:, o2 : o2 + w],
                                pt[:, col : col + w],
                            )
                        else:
                            cut = min(o2 + w, lo)
                            peng.tensor_copy(
                                acc2[:, o2:cut], pt[:, col : col + (cut - o2)]
                            )
                            if o2 + w > lo:
                                peng.tensor_add(
                                    acc2[:, lo : o2 + w], acc2[:, lo : o2 + w],
                                    pt[:, col + (lo - o2) : col + w],
                                )
                            stt["cover2"] = o2

                def do_o():
                    # O-matmuls (PE)
                    for col, h, s, j, off, w, diag, pf, sd in blocks:
                        stt = strip_states[(h, s)]
                        nc.tensor.matmul(
                            stt["o_ps"][:, off : off + w],
                            lhsT=qkv_tiles[h][
                                :,
                                QOFF + 2 * S + KB * j : QOFF + 2 * S + KB * (j + 1),
                            ],
                            rhs=pt[:, col : col + w],
                            start=stt["ocount"] == 0,
                            stop=stt["ocount"] == stt["nmm"] - 1,
                        )
                        stt["ocount"] += 1

                # at the tail the O->evacuate->store chain is critical:
                # emit it before the acc adds
                if tail:
                    do_o(), do_acc()
                else:
                    do_acc(), do_o()
                # strip epilogue: deferred one consume cycle (see
                # emit_fins) so its PSUM evacuation is ready when issued
                for col, h, s, j, off, w, diag, pf, sd in blocks:
                    if sd:
                        pending_fin.append((h, s, strip_states.pop((h, s)), tail))

            load_head(0, first=True)
            pend_q = []
            # static per-engine busy estimates (ns) steer which engine
            # exponentiates each group: the ACT table exp is exact and
            # mandatory for groups holding causally-masked (diagonal)
            # blocks; clean below-diagonal groups may use the Schraudolph
            # bit-trick exp on DVE (tensor_scalar, ~1.04ns/col) or Pool
            # (~1.39ns/col) to offload the saturated ACT engine.
            est = {"act": 0.0, "dve": 0.0, "pool": 0.0}
            prev_eng = [None]
            ng = len(groups)
            for gi, (used, blocks) in enumerate(groups):
                for col, h, s, j, off, w, diag, pf, sd in blocks:
                    if pf:
                        load_head(h + 1)
                sg = psg.tile([128, CAP], f32, tag=f"sg{gi % CFG_RING}")
                for col, h, s, j, off, w, diag, pf, sd in blocks:
                    get_state(h, s)
                    qkv_sb = qkv_tiles[h]
                    # strip 0 holds the short causal rows where softmax
                    # weight jitter does NOT average out: keep it bf16 +
                    # exact ACT exp. Everything else runs fp8 DoubleRow at
                    # 2x PE speed. (Head 0's strip 1 also stays bf16 so
                    # the fp8 load may trail the critical first groups.)
                    f8ok = s > 0 and (h > 0 or s >= 2)
                    if f8ok:
                        t8 = qk8_tiles[h]
                        nc.tensor.matmul(
                            sg[:, col : col + w],
                            lhsT=t8[:, :, S + KB * j : S + KB * (j + 1)],
                            rhs=t8[:, :, QS * s + off : QS * s + off + w],
                            start=True,
                            stop=not diag,
                            perf_mode=DoubleRow,
                        )
                    else:
                        nc.tensor.matmul(
                            sg[:, col : col + w],
                            lhsT=qkv_sb[
                                :, QOFF + S + KB * j : QOFF + S + KB * (j + 1)
                            ],
                            rhs=qkv_sb[
                                :, QOFF + QS * s + off : QOFF + QS * s + off + w
                            ],
                            start=True,
                            stop=not diag,
                        )
                    if diag:
                        # fold the causal mask into the raw scores: PSUM-
                        # accumulate -BIG * (k_local > q_local) over the
                        # piece's first 128 columns (its triangle), so the
                        # exp emits exact zeros there and no separate mask
                        # multiply is needed anywhere downstream
                        nc.tensor.matmul(
                            sg[:, col : col + 128],
                            lhsT=neg_eye,
                            rhs=tri_gt,
                            start=False,
                            stop=True,
                        )
                pt = ptpool.tile([128, CAP], bf16, tag=f"pt{gi % CFG_RING}")
                # consume older groups BEFORE emitting this group's exp:
                # on an offloaded (DVE/Pool) exp the engine queues are
                # in-order, and an exp still waiting on PE scores must not
                # block ready mask/add side-work queued behind it
                lag = int(os.environ.get("K_LAG", "3")) if gi < ng - 3 else 1
                while len(pend_q) >= lag:
                    consume(pend_q.pop(0), tail=gi >= ng - 3)
                # side work this group induces (adds on DVE, masks + strip
                # evacuation on Pool)
                # offload needs every block's rows to have >=512 keys:
                # exclude strip-0 groups (short rows -> exact ACT exp)
                clean = (all(b[2] > 0 for b in blocks)
                         and gi < ng - int(os.environ.get("K_TAILACT", "3")))
                cost = {
                    "act": used * 0.833 + 185,
                    "dve": used * 1.04 + 125 + CFG_DVEBIAS,
                }  # ns; per-instruction access penalties included
                pat = int(os.environ.get("K_PAT", "3"))
                if clean and pat:
                    eng = "dve" if gi % pat == pat - 1 else "act"
                elif clean:
                    # capacity-aware rotation: back-to-back groups on one
                    # engine serialize the sg ring, so penalize repeats
                    eng = min(
                        cost,
                        key=lambda e: max(
                            est[x] + (cost[x] if x == e else 0)
                            for x in est
                        ) + (300 if e == prev_eng[0] else 0),
                    )
                else:
                    eng = "act"
                prev_eng[0] = eng
                est[eng] += cost[eng]
                if eng == "act":
                    nc.scalar.activation(
                        pt[:, 0:used], sg[:, 0:used], Exp, scale=SCALE
                    )
                else:
                    veng = nc.vector
                    veng.tensor_scalar(
                        pt[:, 0:used].bitcast(i16), sg[:, 0:used],
                        EXA * SCALE, EXB, Mult, Add,
                    )
                pend_q.append((blocks, pt))
            while pend_q:
                consume(pend_q.pop(0), tail=True)
            emit_fins()
    nc.compile()
    return nc


def get_nc():
    if "nc" not in _nc_cache:
        _nc_cache["nc"] = _build_nc()
    return _nc_cache["nc"]


BIG = 50.0 / SCALE  # raw-score causal bias; exp((s-BIG)*SCALE) ~ 0


def _build_const():
    import ml_dtypes

    dk = np.arange(128)[:, None]
    c = np.arange(128)[None, :]
    cst = np.zeros((128, 256), ml_dtypes.bfloat16)
    cst[:, 0:128] = (dk > c).astype(ml_dtypes.bfloat16)
    cst[:, 128:256] = -BIG * np.eye(128, dtype=np.float32)
    return cst


def make_in_maps(qkv):
    import ml_dtypes

    qkv = np.asarray(qkv, dtype=np.float32)
    cst = _build_const()
    in_maps = []
    for core in range(NCORES):
        qkvT = np.empty((HPC, 128, 3 * S + 256), ml_dtypes.bfloat16)
        qk8 = np.empty((HPC, 64, 2, 2 * S), ml_dtypes.float8_e4m3)
        qkvT[:, :, 0:256] = cst
        for i in range(HPC):
            bh = core * HPC + i
            b, h = bh // H, bh % H
            qT = qkv[b, :, 0, h, :].T
            kT = qkv[b, :, 1, h, :].T
            qkvT[i, :, 256 : 256 + S] = qT
            qkvT[i, :, 256 + S : 256 + 2 * S] = kT
            qkvT[i, :, 256 + 2 * S :] = (
                qkv[b, :, 2, h, :]
                .reshape(S // KB, KB, D)
                .transpose(1, 0, 2)
                .reshape(KB, S)
            )
            qk8[i, :, 0, 0:S] = qT[0:64]
            qk8[i, :, 1, 0:S] = qT[64:128]
            qk8[i, :, 0, S:] = kT[0:64]
            qk8[i, :, 1, S:] = kT[64:128]
        in_maps.append({"qkvT": qkvT, "qk8": qk8})
    return in_maps


def assemble_out(results):
    out = np.empty((B, S, H, D), np.float32)
    for core in range(NCORES):
        obc = np.asarray(results[core]["obT"], dtype=np.float32)
        for i in range(HPC):
            bh = core * HPC + i
            b, h = bh // H, bh % H
            ob = obc[i].reshape(128, NSTRIP, SW)
            oT = ob[:, :, 0:QS].reshape(128, S)         # unnormalized O^T
            den = ob[:, :, QS : 2 * QS].sum(axis=0)     # full-block sums
            den[:, 128:] += ob[:, :, 2 * QS :].sum(axis=0)  # partial pieces
            den = den.reshape(S)
            out[b, :, h, :] = (oT / den[None, :]).T
    return out


def kernel(qkv):
    from concourse.bass_utils import run_bass_kernel_spmd

    in_maps = make_in_maps(qkv)
    nc = get_nc()
    res = run_bass_kernel_spmd(nc, in_maps, list(range(NCORES)))
    return assemble_out(res.results)

CAP = CAP_[CFG_RING]
